# revision 1
# baseline (speedup 1.0000x reference)
"""BinSAGE (3-layer bipartite GraphSAGE, mean aggregation) on 8 Trainium2 NeuronCores.

Strategy (graph partition by destination):
- Each layer's target nodes are sharded contiguously across the 8 cores; each core
  holds the edges whose dst falls in its partition.
- Per core, target nodes are degree-sorted and packed into groups of 128 (one SBUF
  partition per node).  Each group's neighbor lists are padded to the group max
  degree (ELL format) with pointers at a known all-zeros row, giving a fully
  regular gather + segment-mean:
    one indirect DMA gathers [128, (1+D)*C] features (slot 0 = the root/target
    node, slots 1..D = neighbors), a strided VectorE reduce sums the neighbor
    slots, a per-partition scale by 1/max(deg,1) forms the mean.
- The SAGE transform runs on the PE: transpose(mean), transpose(root) via
  identity matmuls, then out = meanT^T @ Wl + rootT^T @ Wr + mask^T @ b
  accumulated in PSUM (the mask zeroes rows of padding nodes so a padding row
  of the layer output doubles as the next layer's zero row).
- Layer 0 gathers from a per-core compacted copy of x (only rows referenced by
  that core's edges).  Between layers, slices are AllGather'd into a shared
  full table that the next layer's gathers (and its root loads) index into.
- Final log_softmax computed on-chip; host just concatenates + un-permutes rows.
"""

import numpy as np

import concourse.bass as bass
import concourse.bacc as bacc
import concourse.mybir as mybir
import concourse.tile as tile
from concourse.masks import make_identity

NC = 8
P = 128
F32 = mybir.dt.float32
I32 = mybir.dt.int32

# SBUF budget per partition for one gather chunk (bytes); chunks are ELL groups
# batched into a single indirect DMA to amortize SWDGE fixed overhead.
CHUNK_BYTES = 16384


# --------------------------------------------------------------------------- #
# host-side planning
# --------------------------------------------------------------------------- #

def _layer_plan(src, dst, n_tgt):
    """Partition edges by dst; degree-sort each core's targets; shared schedule."""
    s = n_tgt // NC
    assert s * NC == n_tgt
    G = -(-s // P)
    SP = G * P
    per_core = []
    for k in range(NC):
        lo = k * s
        m = (dst >= lo) & (dst < lo + s)
        ds = (dst[m] - lo).astype(np.int64)
        ss = src[m].astype(np.int64)
        deg = np.bincount(ds, minlength=s).astype(np.int64)
        order = np.argsort(-deg, kind="stable")
        e_order = np.argsort(ds, kind="stable")
        csr_src = ss[e_order]
        starts = np.zeros(s + 1, np.int64)
        np.cumsum(deg, out=starts[1:])
        per_core.append((deg, order, csr_src, starts))
    D = []
    for g in range(G):
        i = g * P
        d = max(int(pc[0][pc[1][i]]) for pc in per_core) if i < s else 0
        D.append(d)
    # orig target id -> padded global row of this layer's output table
    row_of_tgt = np.empty(n_tgt, np.int64)
    for k in range(NC):
        order = per_core[k][1]
        row_of_tgt[k * s + order] = k * SP + np.arange(s)
    return dict(s=s, G=G, SP=SP, D=D, per_core=per_core, row_of_tgt=row_of_tgt)


def _core_tables(plan, k, src_row_map, zero_row, fuse_root=True):
    """Build core k's ELL matrix (slot 0 of each group = root row when
    fuse_root, else roots are returned separately), plus rcp [P, G],
    msk [G*P] and the group slot offsets."""
    s, G, D = plan["s"], plan["G"], plan["D"]
    deg, order, csr_src, starts = plan["per_core"][k]
    csr_rows = src_row_map[csr_src] if len(csr_src) else csr_src
    rs = 1 if fuse_root else 0
    tot_slots = rs * G + sum(D)
    ell = np.full((P, tot_slots), zero_row, np.int64)
    roots = np.full(G * P, zero_row, np.int64)
    rcp = np.zeros((P, G), np.float32)
    msk = np.zeros(G * P, np.float32)
    offs = []
    off = 0
    for g in range(G):
        Dg = D[g]
        offs.append(off)
        i0 = g * P
        n = min(P, s - i0)
        nodes = order[i0:i0 + n]
        dg = deg[nodes]
        if fuse_root:
            ell[:n, off] = src_row_map[nodes + k * s]
        roots[g * P:g * P + n] = src_row_map[nodes + k * s]
        if Dg > 0:
            j = np.arange(Dg)[None, :]
            pos = starts[nodes][:, None] + j
            valid = j < dg[:, None]
            pos = np.where(valid, pos, 0)
            tab = np.where(valid, csr_rows[pos], zero_row)
            ell[:n, off + rs:off + rs + Dg] = tab
        rcp[:n, g] = 1.0 / np.maximum(dg, 1)
        msk[g * P:g * P + n] = 1.0
        off += rs + Dg
    return ell, rcp, msk, offs, roots


def _plan_all(x, src0, dst0, src1, dst1, src2, dst2, n1, n2, n3):
    n0, in_c = x.shape
    p0 = _layer_plan(src0, dst0, n1)
    p1 = _layer_plan(src1, dst1, n2)
    p2 = _layer_plan(src2, dst2, n3)

    # layer 0: per-core compacted x; the core's root-feature rows are appended
    # contiguously in group order at RB0 so the root load is a plain DMA.
    ident = np.arange(n0 + 1, dtype=np.int64)
    raw0 = [_core_tables(p0, k, ident, n0, fuse_root=False) for k in range(NC)]
    xt_list, ell0_list = [], []
    for k in range(NC):
        ell = raw0[k][0]
        uniq = np.unique(ell)
        remap = np.searchsorted(uniq, ell)
        ell0_list.append(remap.astype(np.int64))
        xt_list.append(uniq)
    RB0 = max(len(u) for u in xt_list) + 1  # zero row guaranteed in [0, RB0)
    XR = RB0 + p0["SP"]
    xts = []
    for k in range(NC):
        u = xt_list[k]
        xk = np.zeros((XR, in_c), np.float32)
        real = u < n0
        xk[:len(u)][real] = x[u[real]]
        roots = raw0[k][4]
        rreal = roots < n0
        xk[RB0:][rreal] = x[roots[rreal]]
        xts.append(xk)

    # layers 1 & 2 gather from the gathered padded tables
    assert p0["s"] < p0["SP"] and p1["s"] < p1["SP"], (
        "need a padding row to serve as the zero row"
    )
    zr1 = p0["s"]  # first padding row of core 0's slice (output rows are masked to 0)
    zr2 = p1["s"]
    raw1 = [_core_tables(p1, k, p0["row_of_tgt"], zr1) for k in range(NC)]
    raw2 = [_core_tables(p2, k, p1["row_of_tgt"], zr2) for k in range(NC)]

    def pack(raws, ells=None):
        ells = ells if ells is not None else [r[0] for r in raws]
        return dict(
            ell=[e.astype(np.int32) for e in ells],
            rcp=[r[1] for r in raws],
            msk=[r[2] for r in raws],
            offs=raws[0][3],
        )

    return dict(
        p0=p0, p1=p1, p2=p2, XR=XR, RB0=RB0, xts=xts,
        t0=pack(raw0, ell0_list), t1=pack(raw1), t2=pack(raw2),
    )


# --------------------------------------------------------------------------- #
# device kernel
# --------------------------------------------------------------------------- #

def _chunk_groups(D, cin, rs):
    """Batch consecutive groups into gather chunks within the SBUF budget."""
    chunks, cur, slots = [], [], 0
    cap = max(1, CHUNK_BYTES // (cin * 4))
    for g, d in enumerate(D):
        if cur and slots + d + rs > cap:
            chunks.append(cur)
            cur, slots = [], 0
        cur.append(g)
        slots += d + rs
    if cur:
        chunks.append(cur)
    return chunks


def _emit_layer(nc, tc, lay, feat_ap, out_slice, wl_tiles, wr_tiles, b_tile,
                cin, cout, log_softmax=False, out_ext=None):
    """Emit one SAGE layer inside its own tile pools (freed at layer end)."""
    with (
        tc.tile_pool(name=f"lay{lay['i']}_sbuf", bufs=1) as sbuf,
        tc.tile_pool(name=f"lay{lay['i']}_psum", bufs=1, space="PSUM") as psum,
    ):
        _emit_layer_inner(nc, tc, sbuf, psum, lay, feat_ap, out_slice, wl_tiles,
                          wr_tiles, b_tile, cin, cout, log_softmax, out_ext)


def _emit_layer_inner(nc, tc, sbuf, psum, lay, feat_ap, out_slice, wl_tiles,
                      wr_tiles, b_tile, cin, cout, log_softmax, out_ext):
    G, D, offs = lay["G"], lay["D"], lay["offs"]
    rb = lay.get("rb")          # root-base row in the feature table (layer 0)
    rs = 0 if rb is not None else 1
    ident = lay["ident"]
    nt = -(-cin // P)  # transpose chunks along the feature dim

    slots_tot = rs * G + sum(D)
    ell_sb = sbuf.tile([P, slots_tot], I32, name=f"ell_sb_{lay['i']}")
    nc.sync.dma_start(out=ell_sb[:], in_=lay["ell"][:, :])
    # rcp is consumed by TensorScalarPtr, whose ISA struct has a single sync-wait
    # slot; bounce it through a DVE copy so those reads are same-engine ordered.
    rcp_raw = sbuf.tile([P, G], F32, name=f"rcp_raw_{lay['i']}")
    nc.sync.dma_start(out=rcp_raw[:], in_=lay["rcp"][:, :])
    rcp_sb = sbuf.tile([P, G], F32, name=f"rcp_sb_{lay['i']}")
    nc.vector.tensor_copy(rcp_sb[:], rcp_raw[:])
    msk_sb = sbuf.tile([1, G * P], F32, name=f"msk_sb_{lay['i']}")
    nc.sync.dma_start(out=msk_sb[:], in_=lay["msk"][None, :])

    for ch in _chunk_groups(D, cin, rs):
        c0 = offs[ch[0]]
        c_slots = sum(D[g] + rs for g in ch)
        msg = sbuf.tile([P, max(c_slots, 1) * cin], F32, tag=f"msg{lay['i']}",
                        bufs=6, name=f"msg_{lay['i']}_{ch[0]}")
        # HW indirect DMA consumes exactly one index per destination partition
        # row, so gather one ELL slot column (128 rows) per instruction.
        for j in range(c_slots):
            g = nc.gpsimd.indirect_dma_start(
                out=msg[:, j * cin:(j + 1) * cin],
                out_offset=None,
                in_=feat_ap,
                in_offset=bass.IndirectOffsetOnAxis(
                    ap=ell_sb[:, c0 + j:c0 + j + 1], axis=0,
                ),
            )
            if j % 2:
                g.ins.queue = "qPoolDynamic1"  # spread issue over both SWDGE queues
        for g in ch:
            Dg = D[g]
            base = (offs[g] - c0) * cin
            if rs:
                root_ap = msg[:, base:base + cin]
            else:
                # roots are contiguous rows [rb + g*P, rb + (g+1)*P) of the table
                root_t = sbuf.tile([P, cin], F32, tag="root0", bufs=4,
                                   name=f"root_{lay['i']}_{g}")
                nc.sync.dma_start(out=root_t[:],
                                  in_=feat_ap[rb + g * P:rb + (g + 1) * P, :])
                root_ap = root_t[:]
            mean = sbuf.tile([P, cin], F32, tag=f"mean{lay['i']}", bufs=3,
                             name=f"mean_{lay['i']}_{g}")
            if Dg > 0:
                nc.vector.tensor_reduce(
                    out=mean[:],
                    in_=msg[:, base + rs * cin:base + (rs + Dg) * cin]
                        .rearrange("p (j c) -> p c j", c=cin),
                    axis=mybir.AxisListType.X,
                    op=mybir.AluOpType.add,
                )
                nc.vector.tensor_scalar_mul(mean[:], mean[:], rcp_sb[:, g:g + 1])
            else:
                nc.vector.memset(mean[:], 0.0)

            h_ps = psum.tile([P, cout], F32, tag="h_ps", bufs=2,
                             name=f"h_ps_{lay['i']}_{g}")
            first = True
            for pth, tin in ((0, mean[:]), (1, root_ap)):
                for t in range(nt):
                    ct = min(P, cin - t * P)
                    tp = psum.tile([ct, P], F32, tag="tp", bufs=4,
                                   name=f"tp_{lay['i']}_{g}_{pth}_{t}")
                    nc.tensor.transpose(
                        out=tp[:], in_=tin[:, t * P:t * P + ct], identity=ident[:],
                    )
                    tps = sbuf.tile([ct, P], F32, tag="tps", bufs=4,
                                    name=f"tps_{lay['i']}_{g}_{pth}_{t}")
                    nc.scalar.copy(tps[:], tp[:])
                    w = (wl_tiles if pth == 0 else wr_tiles)[t]
                    nc.tensor.matmul(h_ps[:], lhsT=tps[:], rhs=w[:],
                                     start=first, stop=False)
                    first = False
            nc.tensor.matmul(h_ps[:], lhsT=msk_sb[:, g * P:(g + 1) * P],
                             rhs=b_tile[:], start=False, stop=True)

            o_sb = sbuf.tile([P, cout], F32, tag=f"o{lay['i']}", bufs=3,
                             name=f"o_{lay['i']}_{g}")
            if not log_softmax:
                nc.vector.tensor_copy(o_sb[:], h_ps[:])
                nc.sync.dma_start(out=out_slice[g * P:(g + 1) * P, :], in_=o_sb[:])
            else:
                # hop PSUM->SBUF on DVE first: downstream Ptr-variant ops
                # (Exp with AP bias, tensor_scalar) get single-sem waits.
                h_sb = sbuf.tile([P, cout], F32, tag="h_sb", bufs=2,
                                 name=f"h_sb_{g}")
                nc.vector.tensor_copy(h_sb[:], h_ps[:])
                negm = sbuf.tile([P, 1], F32, tag="negm", bufs=2,
                                 name=f"negm_{g}")
                nc.vector.tensor_reduce(out=negm[:], in_=h_sb[:],
                                        axis=mybir.AxisListType.X,
                                        op=mybir.AluOpType.max, negate=True)
                esum = sbuf.tile([P, 1], F32, tag="esum", bufs=2, name=f"esum_{g}")
                etile = sbuf.tile([P, cout], F32, tag="etile", bufs=2,
                                  name=f"etile_{g}")
                nc.scalar.activation(out=etile[:], in_=h_sb[:],
                                     func=mybir.ActivationFunctionType.Exp,
                                     bias=negm[:], scale=1.0, accum_out=esum[:])
                lns_a = sbuf.tile([P, 1], F32, tag="lns_a", bufs=2,
                                  name=f"lns_a_{g}")
                nc.scalar.activation(out=lns_a[:], in_=esum[:],
                                     func=mybir.ActivationFunctionType.Ln)
                nc.vector.tensor_scalar(
                    out=o_sb[:], in0=h_sb[:], scalar1=negm[:], scalar2=lns_a[:],
                    op0=mybir.AluOpType.add, op1=mybir.AluOpType.subtract,
                )
                nc.sync.dma_start(out=out_ext[g * P:(g + 1) * P, :], in_=o_sb[:])


def _build_nc(meta):
    """meta: shapes + degree schedules (identical across cores -> one SPMD program)."""
    in_c, hid, out_c = meta["in_c"], meta["hid"], meta["out_c"]
    nc = bacc.Bacc("TRN2", target_bir_lowering=False, debug=False,
                   num_devices=NC, num_swdge_queues=2)

    xt = nc.dram_tensor("xt", [meta["XR"], in_c], F32, kind="ExternalInput")
    dram_in = {}
    for i, (g, d) in enumerate(((meta["G0"], meta["D0"]), (meta["G1"], meta["D1"]),
                               (meta["G2"], meta["D2"]))):
        slots = (0 if i == 0 else g) + sum(d)  # layer 0 roots live in the table
        dram_in[f"ell{i}"] = nc.dram_tensor(f"ell{i}", [P, slots], I32,
                                            kind="ExternalInput")
        dram_in[f"rcp{i}"] = nc.dram_tensor(f"rcp{i}", [P, g], F32,
                                            kind="ExternalInput")
        dram_in[f"msk{i}"] = nc.dram_tensor(f"msk{i}", [g * P], F32,
                                            kind="ExternalInput")
    wl0 = nc.dram_tensor("wl0", [in_c, hid], F32, kind="ExternalInput")
    wr0 = nc.dram_tensor("wr0", [in_c, hid], F32, kind="ExternalInput")
    b0 = nc.dram_tensor("b0", [hid], F32, kind="ExternalInput")
    wl1 = nc.dram_tensor("wl1", [hid, hid], F32, kind="ExternalInput")
    wr1 = nc.dram_tensor("wr1", [hid, hid], F32, kind="ExternalInput")
    b1 = nc.dram_tensor("b1", [hid], F32, kind="ExternalInput")
    wl2 = nc.dram_tensor("wl2", [hid, out_c], F32, kind="ExternalInput")
    wr2 = nc.dram_tensor("wr2", [hid, out_c], F32, kind="ExternalInput")
    b2 = nc.dram_tensor("b2", [out_c], F32, kind="ExternalInput")
    out = nc.dram_tensor("out", [meta["G2"] * P, out_c], F32, kind="ExternalOutput")

    with tile.TileContext(nc) as tc:
        with (
            tc.tile_pool(name="const", bufs=1) as const,
            tc.tile_pool(name="dram", bufs=1, space="DRAM") as dram,
        ):
            ident = const.tile([P, P], F32)
            make_identity(nc, ident[:])

            def load_w(t, rows, cols):
                nt = -(-rows // P)
                tiles = []
                for i in range(nt):
                    ct = min(P, rows - i * P)
                    w_sb = const.tile([ct, cols], F32, name=f"w_{t.name}_{i}")
                    nc.sync.dma_start(out=w_sb[:], in_=t[i * P:i * P + ct, :])
                    tiles.append(w_sb)
                return tiles

            wl0_t, wr0_t = load_w(wl0, in_c, hid), load_w(wr0, in_c, hid)
            wl1_t, wr1_t = load_w(wl1, hid, hid), load_w(wr1, hid, hid)
            wl2_t, wr2_t = load_w(wl2, hid, out_c), load_w(wr2, hid, out_c)
            b0_sb = const.tile([1, hid], F32)
            nc.sync.dma_start(out=b0_sb[:], in_=b0[None, :])
            b1_sb = const.tile([1, hid], F32)
            nc.sync.dma_start(out=b1_sb[:], in_=b1[None, :])
            b2_sb = const.tile([1, out_c], F32)
            nc.sync.dma_start(out=b2_sb[:], in_=b2[None, :])

            h1_slice = dram.tile([meta["G0"] * P, hid], F32)
            h1_full = dram.tile([NC * meta["G0"] * P, hid], F32,
                                addr_space="Shared")
            h2_slice = dram.tile([meta["G1"] * P, hid], F32)
            h2_full = dram.tile([NC * meta["G1"] * P, hid], F32,
                                addr_space="Shared")

            lay0 = dict(i=0, G=meta["G0"], D=meta["D0"], offs=meta["OFF0"],
                        ident=ident, ell=dram_in["ell0"], rcp=dram_in["rcp0"],
                        msk=dram_in["msk0"], rb=meta["RB0"])
            _emit_layer(nc, tc, lay0, xt[:, :], h1_slice, wl0_t, wr0_t,
                        b0_sb, in_c, hid)
            nc.gpsimd.collective_compute(
                "AllGather", mybir.AluOpType.bypass,
                replica_groups=[list(range(NC))],
                ins=[h1_slice[:]], outs=[h1_full[:]],
            )

            lay1 = dict(i=1, G=meta["G1"], D=meta["D1"], offs=meta["OFF1"],
                        ident=ident, ell=dram_in["ell1"], rcp=dram_in["rcp1"],
                        msk=dram_in["msk1"])
            _emit_layer(nc, tc, lay1, h1_full[:], h2_slice, wl1_t, wr1_t,
                        b1_sb, hid, hid)
            nc.gpsimd.collective_compute(
                "AllGather", mybir.AluOpType.bypass,
                replica_groups=[list(range(NC))],
                ins=[h2_slice[:]], outs=[h2_full[:]],
            )

            lay2 = dict(i=2, G=meta["G2"], D=meta["D2"], offs=meta["OFF2"],
                        ident=ident, ell=dram_in["ell2"], rcp=dram_in["rcp2"],
                        msk=dram_in["msk2"])
            _emit_layer(nc, tc, lay2, h2_full[:], None, wl2_t, wr2_t,
                        b2_sb, hid, out_c, log_softmax=True, out_ext=out)
    nc.finalize()
    return nc


# --------------------------------------------------------------------------- #
# entry point
# --------------------------------------------------------------------------- #

def _prepare(x, src0, dst0, src1, dst1, src2, dst2, n1, n2, n3,
             Wl0, Wr0, b0, Wl1, Wr1, b1, Wl2, Wr2, b2):
    x = np.asarray(x, np.float32)
    plan = _plan_all(x, np.asarray(src0), np.asarray(dst0), np.asarray(src1),
                     np.asarray(dst1), np.asarray(src2), np.asarray(dst2),
                     int(n1), int(n2), int(n3))
    p0, p1, p2 = plan["p0"], plan["p1"], plan["p2"]
    meta = dict(
        in_c=x.shape[1], hid=Wl0.shape[1], out_c=Wl2.shape[1], XR=plan["XR"],
        G0=p0["G"], D0=p0["D"], OFF0=plan["t0"]["offs"], RB0=plan["RB0"],
        G1=p1["G"], D1=p1["D"], OFF1=plan["t1"]["offs"],
        G2=p2["G"], D2=p2["D"], OFF2=plan["t2"]["offs"],
    )
    in_maps = []
    for k in range(NC):
        m = dict(xt=plan["xts"][k].astype(np.float32))
        for i, t in enumerate((plan["t0"], plan["t1"], plan["t2"])):
            m[f"ell{i}"] = np.ascontiguousarray(t["ell"][k])
            m[f"rcp{i}"] = np.ascontiguousarray(t["rcp"][k])
            m[f"msk{i}"] = np.ascontiguousarray(t["msk"][k])
        m.update(
            wl0=np.asarray(Wl0, np.float32), wr0=np.asarray(Wr0, np.float32),
            b0=np.asarray(b0, np.float32),
            wl1=np.asarray(Wl1, np.float32), wr1=np.asarray(Wr1, np.float32),
            b1=np.asarray(b1, np.float32),
            wl2=np.asarray(Wl2, np.float32), wr2=np.asarray(Wr2, np.float32),
            b2=np.asarray(b2, np.float32),
        )
        in_maps.append(m)
    return plan, meta, in_maps


def _assemble(plan, outs):
    full = np.concatenate(outs, axis=0)  # [NC * G2 * P, out_c] padded rows
    return np.ascontiguousarray(full[plan["p2"]["row_of_tgt"]])


def kernel(**inputs) -> np.ndarray:
    from concourse.bass_utils import run_bass_kernel_spmd

    plan, meta, in_maps = _prepare(**inputs)
    nc = _build_nc(meta)
    res = run_bass_kernel_spmd(nc, in_maps, core_ids=list(range(NC)))
    outs = [res.results[k]["out"] for k in range(NC)]
    return _assemble(plan, outs)

